# revision 1
# baseline (speedup 1.0000x reference)
"""MLA (multi-head latent attention) distributed Bass kernel for TRN2.

Full inputs in / full output out. Sharding: 8 cores = 2 batches x 4 head-groups
(4 heads each), per the head/batch-parallel hint: W_uq/W_uk/W_uv split
column-wise and W_o row-wise over heads; each core's [B,H,S,S] score slice is
local. Host work is limited to weight folding (W_dq@W_uq), fp8/layout packing,
and summing the four per-core W_o partials per batch.

Structure (one core):
  - Q projection as fp8e4m3 DoubleRow matmuls from a host-packed x^T copy
    (score noise from fp8 stays absolute-small because scores are tiny; the
    folded W_dq@W_uq is host-scaled by AQ_SCALE into fp8's normal range).
  - ckv/K/V in bf16 (V and the W_o path cannot take fp8 noise).
  - Causal attention in a transposed [key, query] layout: scores^T from one
    DoubleRow fp8 matmul per (key-block, query-pair), exp on ScalarE with
    1/(sqrt(dk)*AQ_SCALE) folded into the activation scale, causal diagonal
    masked by a GpSimd multiply, and the softmax denominator accumulated free
    via a 64-wide ones block in the V stationary (psum rows 64:128).
  - One global QK/exp stream with the AV stream lagging LAG steps, pipelined
    across segments in two phases: all heads' query-chunk-pair 0 (exp-light,
    carries the remaining projection work), then all heads' pair 1 (exp-heavy,
    runs lean with W_o chunks 0/1 interleaved). ScalarE is the bottleneck
    engine; projections/V/W_o ride in its shadow on PE/DVE/GpSimd.
  - W_o tail (chunks 2/3) with psum evacuation alternating DVE/ScalarE and
    four psum slots in flight so PE paces the tail.
"""

import math
import numpy as np
import ml_dtypes

import concourse.bass as bass
import concourse.bacc as bacc
import concourse.mybir as mybir
import concourse.tile as tile
from concourse import bass_utils

BF16 = ml_dtypes.bfloat16
F8 = ml_dtypes.float8_e4m3

D_MODEL = 1024
N_HEADS = 16
D_K = 64
D_C = 256
B, S = 2, 2048

NH = 4          # heads per core
CH = 512        # query chunk (psum bank)
NCH = S // CH   # 4 query chunks
P = 128
NKB = S // P    # 16 key blocks
INV_SQRT_DK = 1.0 / math.sqrt(D_K)
# the folded W_dq@W_uq entries (~0.01) sit below fp8e4m3's min normal
# (2^-6); scale them into the normal range and undo in the exp scale
AQ_SCALE = 64.0

_cached = None


def build_kernel():
    nc = bacc.Bacc("TRN2", debug=False, num_devices=8)
    dt = mybir.dt
    EXP = mybir.ActivationFunctionType.Exp
    DR = mybir.MatmulPerfMode.DoubleRow
    NKD = D_MODEL // P  # 8 d_model blocks

    xT_d = nc.dram_tensor("xT", [D_MODEL, S], dt.bfloat16, kind="ExternalInput")
    # fp8 copy of x^T packed for DoubleRow: [p, g, s, q] = x^T[256g+128s+p, q]
    xP_d = nc.dram_tensor("xP", [P, 4, 2, S], dt.float8e4, kind="ExternalInput")
    aqp_d = nc.dram_tensor("aqp", [P, 4, 2, NH * D_K], dt.float8e4,
                           kind="ExternalInput")
    wdkv_d = nc.dram_tensor("wdkv", [P, NKD, D_C], dt.bfloat16, kind="ExternalInput")
    wuk_d = nc.dram_tensor("wuk", [P, D_C // P, NH * D_K], dt.bfloat16, kind="ExternalInput")
    wuv_d = nc.dram_tensor("wuv", [P, D_C // P, NH * D_K], dt.bfloat16, kind="ExternalInput")
    wo_d = nc.dram_tensor("wo", [2, P, D_MODEL], dt.bfloat16, kind="ExternalInput")
    tri_d = nc.dram_tensor("tri", [P, P], dt.bfloat16, kind="ExternalInput")
    # output: y^T = (x @ ... @ W_o)^T in [m, q] layout
    yT_d = nc.dram_tensor("yT", [D_MODEL, S], dt.bfloat16, kind="ExternalOutput")

    with tile.TileContext(nc) as tc:
        with (
            tc.tile_pool(name="const", bufs=1) as const,
            tc.tile_pool(name="acts", bufs=1) as acts,
            tc.tile_pool(name="exps", bufs=2) as exps,
            tc.tile_pool(name="work", bufs=8) as work,
            tc.tile_pool(name="ps", bufs=2, space="PSUM") as ps,
            tc.tile_pool(name="psav", bufs=2, space="PSUM") as psa,
            tc.tile_pool(name="psqk", bufs=2, space="PSUM") as psqk,
        ):
            xTv = xT_d.ap().rearrange("(n p) s -> p n s", p=P)
            # warm the exp table set during the initial DMA wait (the lazy
            # ACT_TABLE_LOAD otherwise stalls the first real softmax ~2.7us)
            warm = work.tile([1, 1], dt.float32, tag="warm")
            nc.gpsimd.memset(warm[:], 0.0)
            nc.scalar.activation(warm[:], warm[:],
                                 mybir.ActivationFunctionType.Exp)
            # loads, first-needed first; the Q projection only needs 0.6 MB
            # (aqp + half of xP), so it goes first; xT streams in quarters
            aqp = const.tile([P, 4, 2, NH * D_K], dt.float8e4, tag="aqp")
            nc.sync.dma_start(aqp[:], aqp_d.ap())
            xP = const.tile([P, 4, 2, S], dt.float8e4, tag="xP")
            for q in range(2):
                nc.sync.dma_start(xP[:, :, :, q * CH:(q + 1) * CH],
                                  xP_d.ap()[:, :, :, q * CH:(q + 1) * CH])
            wdkv = const.tile([P, NKD, D_C], dt.bfloat16, tag="wdkv")
            nc.sync.dma_start(wdkv[:], wdkv_d.ap())
            wuk = const.tile([P, D_C // P, NH * D_K], dt.bfloat16, tag="wuk")
            nc.sync.dma_start(wuk[:], wuk_d.ap())
            xT = [const.tile([P, S], dt.bfloat16, name=f"xT{n}", tag=f"xT{n}")
                  for n in range(NKD)]
            for n in range(NKD):
                nc.sync.dma_start(xT[n][:, 0:CH], xTv[:, n, 0:CH])
            tri = const.tile([P, P], dt.bfloat16, tag="tri")
            nc.sync.dma_start(tri[:], tri_d.ap())
            wuv = const.tile([P, D_C // P, NH * D_K], dt.bfloat16, tag="wuv")
            nc.sync.dma_start(wuv[:], wuv_d.ap())
            for n in range(NKD):
                nc.sync.dma_start(xT[n][:, CH:2 * CH], xTv[:, n, CH:2 * CH])
            for q in range(2, 4):
                nc.sync.dma_start(xP[:, :, :, q * CH:(q + 1) * CH],
                                  xP_d.ap()[:, :, :, q * CH:(q + 1) * CH])
            for q in range(2, 4):
                for n in range(NKD):
                    nc.sync.dma_start(xT[n][:, q * CH:(q + 1) * CH],
                                      xTv[:, n, q * CH:(q + 1) * CH])
            wo = []
            for n in range(2):
                t = const.tile([P, D_MODEL], dt.bfloat16, name=f"wo{n}", tag=f"wo{n}")
                nc.sync.dma_start(t[:], wo_d.ap()[n])
                wo.append(t)

            # persistent activations
            ckvT = [acts.tile([P, S], dt.bfloat16, name=f"ckvT{i}", tag=f"ckvT{i}")
                    for i in range(2)]
            # fp8 DoubleRow packing: partition 32h+j = head h dims (2j, 2j+1)
            # in parity slots (host permutes aqp/wuk columns accordingly)
            qTp = acts.tile([P, 2, S], dt.float8e4, name="qTp", tag="qTp")
            kTp = acts.tile([P, 2, S], dt.float8e4, name="kTp", tag="kTp")
            v_sb = [None] * NKB
            outT = [acts.tile([P, S], dt.bfloat16, name=f"outT{m}", tag=f"outT{m}")
                    for m in range(2)]

            # ---- single-psum projection pieces (interleavable) ----
            def emit_ckv(ch, half):
                sl = slice(ch * CH, (ch + 1) * CH)
                pp = ps.tile([P, CH], dt.float32, name="pp", tag="ps")
                for k in range(NKD):
                    nc.tensor.matmul(
                        pp[:], wdkv[:, k, half * P:(half + 1) * P],
                        xT[k][:, sl], start=(k == 0), stop=(k == NKD - 1))
                if ch == 0:
                    # head critical path: ScalarE idles before the first exp
                    # and its psum->sbuf copy is faster than DVE's
                    nc.scalar.copy(ckvT[half][:, sl], pp[:])
                else:
                    nc.vector.tensor_copy(ckvT[half][:, sl], pp[:])

            def emit_k(ch, m):
                sl = slice(ch * CH, (ch + 1) * CH)
                pp = ps.tile([P, CH], dt.float32, name="pp", tag="ps")
                for half in range(2):
                    nc.tensor.matmul(
                        pp[:], wuk[:, half, m * P:(m + 1) * P],
                        ckvT[half][:, sl], start=(half == 0), stop=(half == 1))
                if ch == 0:
                    nc.scalar.copy(kTp[:, m, sl], pp[:])
                else:
                    nc.vector.tensor_copy(kTp[:, m, sl], pp[:])

            def emit_q(ch, m):
                # fp8 DoubleRow: contraction 1024 = 4 groups of 256 (128x2)
                sl = slice(ch * CH, (ch + 1) * CH)
                pp = ps.tile([P, CH], dt.float32, name="pp", tag="ps")
                for g in range(4):
                    nc.tensor.matmul(
                        pp[:], aqp[:, g, :, m * P:(m + 1) * P],
                        xP[:, g, :, sl], start=(g == 0), stop=(g == 3),
                        perf_mode=DR)
                nc.vector.tensor_copy(qTp[:, m, sl], pp[:])

            def emit_v(kb):
                # V in [key, dim]: per head 64 dims + 64-wide ones block
                # (the ones columns replicate the softmax denominator to
                # psum rows 64:128 for free)
                vt = acts.tile([P, NH, 2 * D_K], dt.bfloat16,
                               name=f"v{kb}", tag=f"v{kb}")
                psv = ps.tile([P, NH * D_K], dt.float32, tag="ps")
                for half in range(2):
                    nc.tensor.matmul(
                        psv[:], ckvT[half][:, kb * P:(kb + 1) * P],
                        wuv[:, half, :], start=(half == 0), stop=(half == 1))
                nc.vector.tensor_copy(
                    vt[:, :, 0:D_K],
                    psv[:].rearrange("p (h d) -> p h d", h=NH))
                nc.gpsimd.memset(vt[:, :, D_K:2 * D_K], 1.0)
                v_sb[kb] = vt

            def emit_wo_mb(ch, mb, tailpool=None, eng=None):
                # yT[m, q] = sum_d wo[d, m] outT[d, q]: one (m, q-chunk) block
                sl = slice(ch * CH, (ch + 1) * CH)
                ysb = work.tile([P, CH], dt.bfloat16, tag="ysb")
                if tailpool is not None:
                    pp = tailpool.tile([P, CH], dt.float32, name="pp", tag="qk")
                else:
                    pp = ps.tile([P, CH], dt.float32, name="pp", tag="ps")
                for db in range(2):
                    nc.tensor.matmul(
                        pp[:], wo[db][:, mb * P:(mb + 1) * P],
                        outT[db][:, sl], start=(db == 0), stop=(db == 1))
                if eng == "act":
                    # tail jobs alternate psum evacuation between ScalarE
                    # (idle once the softmax stream drains; Copy lives in
                    # every ACT table set) and DVE, so PE paces the tail
                    nc.scalar.copy(ysb[:], pp[:])
                else:
                    nc.vector.tensor_copy(ysb[:], pp[:])
                nc.sync.dma_start(yT_d.ap()[mb * P:(mb + 1) * P, sl], ysb[:])

            # minimal first projection pass: just enough for QK(h0, kb0..7)
            # and AV(kb0..) -- q/ckv/k for chunks 0,1. Everything else rides
            # in the attention stream as extras.
            for ch in (0, 1):
                for m in range(2):
                    emit_q(ch, m)
            for half in range(2):
                emit_ckv(0, half)
            for m in range(2):
                emit_k(0, m)
            for half in range(2):
                emit_ckv(1, half)
            for m in range(2):
                emit_k(1, m)


            # ---- attention: one global QK/exp stream with the AV stream
            # lagging LAG steps behind, pipelined across segments. Segments
            # run in TWO PHASES: all heads' cp0 (queries 0:1024, exp-light --
            # carries every projection extra), then all heads' cp1 (exp-heavy,
            # runs lean with only W_o interleaved). es tiles are per (cp, kb)
            # so their lifetime stays within a phase. Softmax denominator
            # lives in rows 64:128 of psav. ----
            LAG = 6
            es_t = {}
            psav_heads = [[None] * NCH for _ in range(NH)]

            def emit_qk(h, cp, kb):
                q0 = P * kb       # first valid query for this key block
                pq0 = 1024 * cp   # pair covers q in [pq0, pq0+1024)
                q_h = qTp[32 * h:32 * (h + 1), :, :]
                k_h = kTp[32 * h:32 * (h + 1), :, :]
                lo = max(q0, pq0)
                if (h, cp, kb) not in es_t:
                    es_t[(h, cp, kb)] = exps.tile(
                        [P, pq0 + 2 * CH - lo], dt.bfloat16,
                        name=f"es{cp}_{kb}", tag=f"es{cp}_{kb}")
                es = es_t[(h, cp, kb)]
                pqk = psqk.tile([P, 2 * CH], dt.float32,
                                name="pqk", tag="qk")
                for ch in (2 * cp, 2 * cp + 1):
                    clo = max(q0, ch * CH)
                    if clo >= (ch + 1) * CH:
                        continue
                    nc.tensor.matmul(
                        pqk[:, clo - pq0:(ch + 1) * CH - pq0],
                        k_h[:, :, q0:q0 + P],
                        q_h[:, :, clo:(ch + 1) * CH],
                        start=True, stop=True, perf_mode=DR,
                        tile_position=(32 * h, 0))
                # 1/AQ_SCALE undoes the host-side fp8-range scaling of aqp
                nc.scalar.activation(
                    es[:], pqk[:, lo - pq0:2 * CH],
                    EXP, scale=INV_SQRT_DK / AQ_SCALE)
                if cp == kb // 8:
                    # mask the diagonal [128, 128] triangle (valid f >= p)
                    # on GpSimd -- it idles while DVE is near-saturated
                    nc.gpsimd.tensor_mul(es[:, 0:P], es[:, 0:P], tri[:])

            def emit_av(h, cp, kb):
                q0 = P * kb
                lo = max(q0, 1024 * cp)
                ht, off = divmod(h, 2)
                psav = psav_heads[h]
                for c in (2 * cp, 2 * cp + 1):
                    if kb // 4 > c:
                        continue
                    if psav[c] is None:
                        psav[c] = psa.tile([P, CH], dt.float32,
                                           name="psav", tag="psav")
                    n0 = max(q0 - CH * c, 0)
                    nc.tensor.matmul(
                        psav[c][:, n0:CH], v_sb[kb][:, h, :],
                        es_t[(h, cp, kb)][:, CH * c + n0 - lo:
                                          CH * (c + 1) - lo],
                        start=(kb == touch[(h, c)][0]),
                        stop=(kb == touch[(h, c)][-1]))
                    if kb == touch[(h, c)][-1]:  # chunk done -> normalize
                        rb = work.tile([D_K, CH], dt.float32, tag="rb")
                        nc.vector.reciprocal(
                            rb[:], psav[c][D_K:2 * D_K, :])
                        nc.vector.tensor_mul(
                            outT[ht][off * D_K:(off + 1) * D_K,
                                     c * CH:(c + 1) * CH],
                            psav[c][0:D_K, :], rb[:])

            # segment order: phase 0 = (h, cp0) for all h; phase 1 = (h, cp1).
            # Middle segments run narrow-exp-first so a segment's opening
            # steps never serialize PE behind a wide exp; h0cp0 (extras) and
            # h3cp1 (W_o tail needs early chunk completion) stay ascending.
            segs = [(h, 0) for h in range(NH)] + [(h, 1) for h in range(NH)]

            def kb_order(h, cp):
                # ascending everywhere: any other order makes a chunk's first
                # psum touch a partial-range start=True, which leaves the
                # accumulation in mixed pending-zero state (unmodelable)
                return list(range(8 * cp + 8))

            stream = []
            seg_start = {}
            touch = {}
            for h, cp in segs:
                seg_start[(h, cp)] = len(stream)
                for kb in kb_order(h, cp):
                    for c in (2 * cp, 2 * cp + 1):
                        if kb // 4 <= c:
                            touch.setdefault((h, c), []).append(kb)
                    stream.append((h, cp, kb))

            # extras: projections spread over the cp0 phase; W_o (chunks 0,1
            # ready after h3cp0) spread over the first three cp1 segments;
            # W_o chunk 2 late in h3cp1, chunk 3 after the stream.
            extras = {}

            def put(seg, kb, fn):
                extras.setdefault(seg_start[seg] + kb, []).append(fn)

            for kb in range(8):
                put((0, 0), kb, lambda kb=kb: emit_v(kb))
            p30 = [lambda: emit_q(2, 0), lambda: emit_q(2, 1),
                   lambda: emit_q(3, 0), lambda: emit_q(3, 1)]
            for i, fn in enumerate(p30):
                put((3, 0), i, fn)
            # ch2/3 projections + their V blocks ride in h0cp1, where the
            # exp stream saturates ACT and PE has idle slots
            p01 = [lambda: emit_ckv(2, 0), lambda: emit_ckv(2, 1),
                   lambda: emit_k(2, 0), lambda: emit_k(2, 1),
                   lambda: emit_ckv(3, 0), lambda: emit_ckv(3, 1),
                   lambda: emit_k(3, 0), lambda: emit_k(3, 1),
                   lambda: emit_v(8), lambda: emit_v(9),
                   lambda: emit_v(10), lambda: emit_v(11),
                   lambda: emit_v(12), lambda: emit_v(13),
                   lambda: emit_v(14)]
            for i, fn in enumerate(p01):
                put((0, 1), i, fn)
            put((1, 1), 0, lambda: emit_v(15))

            jobs = [(c, mb) for c in (0, 1) for mb in range(D_MODEL // P)]
            for i, job in enumerate(jobs):
                put((i % 3, 1), 3 * (i // 3) + 1,
                    lambda job=job: emit_wo_mb(job[0], job[1]))



            for t in range(len(stream) + LAG):
                if t < len(stream):
                    emit_qk(*stream[t])
                if t >= LAG:
                    emit_av(*stream[t - LAG])
                for fn in extras.get(t, ()):
                    fn()
            # tail W_o: four psum slots in flight (ps pool + the retired
            # psqk slots) with evacuation alternating DVE/ScalarE, so the
            # tail is paced by PE matmuls rather than copies
            for mb in range(D_MODEL // P):
                emit_wo_mb(2, mb, tailpool=psqk if mb % 2 else None, eng="act")
                emit_wo_mb(3, mb, tailpool=None if mb % 2 else psqk, eng="dve")

    nc.compile()
    return nc


def _fold(w, p=P):
    # [K, M] -> [p, K/p, M] partition-major layout for contiguous DMA
    k, m = w.shape
    return np.ascontiguousarray(w.reshape(k // p, p, m).transpose(1, 0, 2))


def _fold_dr(w, p=P):
    # [K, M] -> [p, K/(2p), 2, M] DoubleRow fp8 packing: contraction row
    # 256g + 128s + p lands at [p, g, s, :]
    k, m = w.shape
    return np.ascontiguousarray(
        w.reshape(k // (2 * p), 2, p, m).transpose(2, 0, 1, 3))


# DoubleRow column permutation: M-col m<128 -> head m//32, dim 2*(m%32);
# m>=128 -> head (m-128)//32, dim 2*((m-128)%32)+1
_PERM = np.array([64 * ((m % 128) // 32) + 2 * (m % 32) + m // 128
                  for m in range(256)])


def _prep_inputs(x, W_dq, W_uq, W_dkv, W_uk, W_uv, W_o):
    tri = np.triu(np.ones((P, P), dtype=np.float32)).astype(BF16)  # f >= p
    in_maps = []
    xPs = []
    for b in range(B):
        xb = np.asarray(x, np.float32)[b]  # [S, D_MODEL]
        xPs.append(np.ascontiguousarray(
            xb.T.reshape(4, 2, P, S).transpose(2, 0, 1, 3)).astype(F8))
    for c in range(8):
        b, hg = divmod(c, 4)
        cs = slice(hg * NH * D_K, (hg + 1) * NH * D_K)
        aq = np.asarray(W_dq, np.float32) @ np.asarray(W_uq, np.float32)[:, cs]
        wuk = np.asarray(W_uk, np.float32)[:, cs]
        in_maps.append({
            "xT": np.ascontiguousarray(np.asarray(x)[b].T).astype(BF16),
            "xP": xPs[b],
            "aqp": _fold_dr(AQ_SCALE * aq[:, _PERM]).astype(F8),
            "wdkv": _fold(np.asarray(W_dkv).astype(BF16)),
            "wuk": _fold(wuk[:, _PERM].astype(BF16)),
            "wuv": _fold(np.asarray(W_uv)[:, cs].astype(BF16)),
            "wo": np.asarray(W_o)[cs, :].astype(BF16).reshape(2, P, D_MODEL),
            "tri": tri,
        })
    return in_maps


def run(inputs, trace=False, **kw):
    global _cached
    if _cached is None:
        _cached = build_kernel()
    in_maps = _prep_inputs(**inputs)
    res = bass_utils.run_bass_kernel_spmd(
        _cached, in_maps, core_ids=list(range(8)), trace=trace, **kw)
    ys = [res.results[c]["yT"].astype(np.float32) for c in range(8)]
    out = np.stack([
        (ys[0] + ys[1] + ys[2] + ys[3]).T,
        (ys[4] + ys[5] + ys[6] + ys[7]).T,
    ]).astype(np.float32)
    return out, res


def kernel(**inputs):
    out, _ = run(inputs)
    return out



# revision 2
# speedup vs baseline: 1.0088x; 1.0088x over previous
"""MLA (multi-head latent attention) distributed Bass kernel for TRN2.

Full inputs in / full output out. Sharding: 8 cores = 2 batches x 4 head-groups
(4 heads each), per the head/batch-parallel hint: W_uq/W_uk/W_uv split
column-wise and W_o row-wise over heads; each core's [B,H,S,S] score slice is
local. Host work is limited to weight folding (W_dq@W_uq), fp8/layout packing,
and summing the four per-core W_o partials per batch.

Structure (one core):
  - Q projection as fp8e4m3 DoubleRow matmuls from a host-packed x^T copy
    (score noise from fp8 stays absolute-small because scores are tiny; the
    folded W_dq@W_uq is host-scaled by AQ_SCALE into fp8's normal range).
  - ckv/K/V in bf16 (V and the W_o path cannot take fp8 noise).
  - Causal attention in a transposed [key, query] layout: scores^T from one
    DoubleRow fp8 matmul per (key-block, query-pair), exp on ScalarE with
    1/(sqrt(dk)*AQ_SCALE) folded into the activation scale, causal diagonal
    masked by a GpSimd multiply, and the softmax denominator accumulated free
    via a 64-wide ones block in the V stationary (psum rows 64:128).
  - One global QK/exp stream with the AV stream lagging LAG steps, pipelined
    across segments in two phases: all heads' query-chunk-pair 0 (exp-light,
    carries the remaining projection work), then all heads' pair 1 (exp-heavy,
    runs lean with W_o chunks 0/1 interleaved). ScalarE is the bottleneck
    engine; projections/V/W_o ride in its shadow on PE/DVE/GpSimd.
  - W_o tail (chunks 2/3) with psum evacuation alternating DVE/ScalarE and
    four psum slots in flight so PE paces the tail.
"""

import math
import numpy as np
import ml_dtypes

import concourse.bass as bass
import concourse.bacc as bacc
import concourse.mybir as mybir
import concourse.tile as tile
from concourse import bass_utils

BF16 = ml_dtypes.bfloat16
F8 = ml_dtypes.float8_e4m3

D_MODEL = 1024
N_HEADS = 16
D_K = 64
D_C = 256
B, S = 2, 2048

NH = 4          # heads per core
CH = 512        # query chunk (psum bank)
NCH = S // CH   # 4 query chunks
P = 128
NKB = S // P    # 16 key blocks
INV_SQRT_DK = 1.0 / math.sqrt(D_K)
# the folded W_dq@W_uq entries (~0.01) sit below fp8e4m3's min normal
# (2^-6); scale them into the normal range and undo in the exp scale
AQ_SCALE = 64.0

_cached = None


def build_kernel():
    nc = bacc.Bacc("TRN2", debug=False, num_devices=8)
    dt = mybir.dt
    EXP = mybir.ActivationFunctionType.Exp
    DR = mybir.MatmulPerfMode.DoubleRow
    NKD = D_MODEL // P  # 8 d_model blocks

    xT_d = nc.dram_tensor("xT", [D_MODEL, S], dt.bfloat16, kind="ExternalInput")
    # fp8 copy of x^T packed for DoubleRow: [p, g, s, q] = x^T[256g+128s+p, q]
    xP_d = nc.dram_tensor("xP", [P, 4, 2, S], dt.float8e4, kind="ExternalInput")
    aqp_d = nc.dram_tensor("aqp", [P, 4, 2, NH * D_K], dt.float8e4,
                           kind="ExternalInput")
    wdkv_d = nc.dram_tensor("wdkv", [P, NKD, D_C], dt.bfloat16, kind="ExternalInput")
    wuk_d = nc.dram_tensor("wuk", [P, D_C // P, NH * D_K], dt.bfloat16, kind="ExternalInput")
    wuv_d = nc.dram_tensor("wuv", [P, D_C // P, NH * D_K], dt.bfloat16, kind="ExternalInput")
    wo_d = nc.dram_tensor("wo", [2, P, D_MODEL], dt.bfloat16, kind="ExternalInput")
    tri_d = nc.dram_tensor("tri", [P, P], dt.bfloat16, kind="ExternalInput")
    # output: y^T = (x @ ... @ W_o)^T in [m, q] layout
    yT_d = nc.dram_tensor("yT", [D_MODEL, S], dt.bfloat16, kind="ExternalOutput")

    with tile.TileContext(nc) as tc:
        with (
            tc.tile_pool(name="const", bufs=1) as const,
            tc.tile_pool(name="acts", bufs=1) as acts,
            tc.tile_pool(name="exps", bufs=2) as exps,
            tc.tile_pool(name="work", bufs=8) as work,
            tc.tile_pool(name="ps", bufs=2, space="PSUM") as ps,
            tc.tile_pool(name="psav", bufs=2, space="PSUM") as psa,
            tc.tile_pool(name="psqk", bufs=2, space="PSUM") as psqk,
        ):
            xTv = xT_d.ap().rearrange("(n p) s -> p n s", p=P)
            # warm the exp table set during the initial DMA wait (the lazy
            # ACT_TABLE_LOAD otherwise stalls the first real softmax ~2.7us)
            warm = work.tile([1, 1], dt.float32, tag="warm")
            nc.gpsimd.memset(warm[:], 0.0)
            nc.scalar.activation(warm[:], warm[:],
                                 mybir.ActivationFunctionType.Exp)
            # loads, first-needed first; the Q projection only needs 0.6 MB
            # (aqp + half of xP), so it goes first; xT streams in quarters
            aqp = const.tile([P, 4, 2, NH * D_K], dt.float8e4, tag="aqp")
            nc.sync.dma_start(aqp[:], aqp_d.ap())
            xP = const.tile([P, 4, 2, S], dt.float8e4, tag="xP")
            for q in range(2):
                nc.sync.dma_start(xP[:, :, :, q * CH:(q + 1) * CH],
                                  xP_d.ap()[:, :, :, q * CH:(q + 1) * CH])
            wdkv = const.tile([P, NKD, D_C], dt.bfloat16, tag="wdkv")
            nc.sync.dma_start(wdkv[:], wdkv_d.ap())
            wuk = const.tile([P, D_C // P, NH * D_K], dt.bfloat16, tag="wuk")
            nc.sync.dma_start(wuk[:], wuk_d.ap())
            xT = [const.tile([P, S], dt.bfloat16, name=f"xT{n}", tag=f"xT{n}")
                  for n in range(NKD)]
            for n in range(NKD):
                nc.sync.dma_start(xT[n][:, 0:CH], xTv[:, n, 0:CH])
            tri = const.tile([P, P], dt.bfloat16, tag="tri")
            nc.sync.dma_start(tri[:], tri_d.ap())
            wuv = const.tile([P, D_C // P, NH * D_K], dt.bfloat16, tag="wuv")
            nc.sync.dma_start(wuv[:], wuv_d.ap())
            for n in range(NKD):
                nc.sync.dma_start(xT[n][:, CH:2 * CH], xTv[:, n, CH:2 * CH])
            for q in range(2, 4):
                nc.sync.dma_start(xP[:, :, :, q * CH:(q + 1) * CH],
                                  xP_d.ap()[:, :, :, q * CH:(q + 1) * CH])
            for q in range(2, 4):
                for n in range(NKD):
                    nc.sync.dma_start(xT[n][:, q * CH:(q + 1) * CH],
                                      xTv[:, n, q * CH:(q + 1) * CH])
            wo = []
            for n in range(2):
                t = const.tile([P, D_MODEL], dt.bfloat16, name=f"wo{n}", tag=f"wo{n}")
                nc.sync.dma_start(t[:], wo_d.ap()[n])
                wo.append(t)

            # persistent activations
            ckvT = [acts.tile([P, S], dt.bfloat16, name=f"ckvT{i}", tag=f"ckvT{i}")
                    for i in range(2)]
            # fp8 DoubleRow packing: partition 32h+j = head h dims (2j, 2j+1)
            # in parity slots (host permutes aqp/wuk columns accordingly)
            qTp = acts.tile([P, 2, S], dt.float8e4, name="qTp", tag="qTp")
            kTp = acts.tile([P, 2, S], dt.float8e4, name="kTp", tag="kTp")
            v_sb = [None] * NKB
            outT = [acts.tile([P, S], dt.bfloat16, name=f"outT{m}", tag=f"outT{m}")
                    for m in range(2)]

            # ---- single-psum projection pieces (interleavable) ----
            def emit_ckv(ch, half):
                sl = slice(ch * CH, (ch + 1) * CH)
                pp = ps.tile([P, CH], dt.float32, name="pp", tag="ps")
                for k in range(NKD):
                    nc.tensor.matmul(
                        pp[:], wdkv[:, k, half * P:(half + 1) * P],
                        xT[k][:, sl], start=(k == 0), stop=(k == NKD - 1))
                if ch == 0:
                    # head critical path: ScalarE idles before the first exp
                    # and its psum->sbuf copy is faster than DVE's
                    nc.scalar.copy(ckvT[half][:, sl], pp[:])
                else:
                    nc.vector.tensor_copy(ckvT[half][:, sl], pp[:])

            def emit_k(ch, m):
                sl = slice(ch * CH, (ch + 1) * CH)
                pp = ps.tile([P, CH], dt.float32, name="pp", tag="ps")
                for half in range(2):
                    nc.tensor.matmul(
                        pp[:], wuk[:, half, m * P:(m + 1) * P],
                        ckvT[half][:, sl], start=(half == 0), stop=(half == 1))
                if ch == 0:
                    nc.scalar.copy(kTp[:, m, sl], pp[:])
                else:
                    nc.vector.tensor_copy(kTp[:, m, sl], pp[:])

            def emit_q(ch, m):
                # fp8 DoubleRow: contraction 1024 = 4 groups of 256 (128x2)
                sl = slice(ch * CH, (ch + 1) * CH)
                pp = ps.tile([P, CH], dt.float32, name="pp", tag="ps")
                for g in range(4):
                    nc.tensor.matmul(
                        pp[:], aqp[:, g, :, m * P:(m + 1) * P],
                        xP[:, g, :, sl], start=(g == 0), stop=(g == 3),
                        perf_mode=DR)
                nc.vector.tensor_copy(qTp[:, m, sl], pp[:])

            def emit_v(kb):
                # V in [key, dim]: per head 64 dims + 64-wide ones block
                # (the ones columns replicate the softmax denominator to
                # psum rows 64:128 for free)
                vt = acts.tile([P, NH, 2 * D_K], dt.bfloat16,
                               name=f"v{kb}", tag=f"v{kb}")
                psv = ps.tile([P, NH * D_K], dt.float32, tag="ps")
                for half in range(2):
                    nc.tensor.matmul(
                        psv[:], ckvT[half][:, kb * P:(kb + 1) * P],
                        wuv[:, half, :], start=(half == 0), stop=(half == 1))
                nc.vector.tensor_copy(
                    vt[:, :, 0:D_K],
                    psv[:].rearrange("p (h d) -> p h d", h=NH))
                nc.gpsimd.memset(vt[:, :, D_K:2 * D_K], 1.0)
                v_sb[kb] = vt

            def emit_wo_mb(ch, mb, tailpool=None, eng=None):
                # yT[m, q] = sum_d wo[d, m] outT[d, q]: one (m, q-chunk) block
                sl = slice(ch * CH, (ch + 1) * CH)
                ysb = work.tile([P, CH], dt.bfloat16, tag="ysb")
                if tailpool is not None:
                    pp = tailpool.tile([P, CH], dt.float32, name="pp", tag="qk")
                else:
                    pp = ps.tile([P, CH], dt.float32, name="pp", tag="ps")
                for db in range(2):
                    nc.tensor.matmul(
                        pp[:], wo[db][:, mb * P:(mb + 1) * P],
                        outT[db][:, sl], start=(db == 0), stop=(db == 1))
                if eng == "act":
                    # tail jobs alternate psum evacuation between ScalarE
                    # (idle once the softmax stream drains; Copy lives in
                    # every ACT table set) and DVE, so PE paces the tail
                    nc.scalar.copy(ysb[:], pp[:])
                else:
                    nc.vector.tensor_copy(ysb[:], pp[:])
                # alternate stores between the HWDGE path (sync) and the
                # SWDGE path (gpsimd): HWDGE is a single-slot serial device
                # (~625ns hold per DMA), so splitting the store train across
                # both issue paths halves the serialized tail
                if mb % 2:
                    nc.gpsimd.dma_start(yT_d.ap()[mb * P:(mb + 1) * P, sl],
                                        ysb[:])
                else:
                    nc.sync.dma_start(yT_d.ap()[mb * P:(mb + 1) * P, sl],
                                      ysb[:])

            # minimal first projection pass: just enough for QK(h0, kb0..7)
            # and AV(kb0..) -- q/ckv/k for chunks 0,1. Everything else rides
            # in the attention stream as extras.
            for ch in (0, 1):
                for m in range(2):
                    emit_q(ch, m)
            for half in range(2):
                emit_ckv(0, half)
            for m in range(2):
                emit_k(0, m)
            for half in range(2):
                emit_ckv(1, half)
            for m in range(2):
                emit_k(1, m)


            # ---- attention: one global QK/exp stream with the AV stream
            # lagging LAG steps behind, pipelined across segments. Segments
            # run in TWO PHASES: all heads' cp0 (queries 0:1024, exp-light --
            # carries every projection extra), then all heads' cp1 (exp-heavy,
            # runs lean with only W_o interleaved). es tiles are per (cp, kb)
            # so their lifetime stays within a phase. Softmax denominator
            # lives in rows 64:128 of psav. ----
            LAG = 6
            es_t = {}
            psav_heads = [[None] * NCH for _ in range(NH)]

            def emit_qk(h, cp, kb):
                q0 = P * kb       # first valid query for this key block
                pq0 = 1024 * cp   # pair covers q in [pq0, pq0+1024)
                q_h = qTp[32 * h:32 * (h + 1), :, :]
                k_h = kTp[32 * h:32 * (h + 1), :, :]
                lo = max(q0, pq0)
                if (h, cp, kb) not in es_t:
                    es_t[(h, cp, kb)] = exps.tile(
                        [P, pq0 + 2 * CH - lo], dt.bfloat16,
                        name=f"es{cp}_{kb}", tag=f"es{cp}_{kb}")
                es = es_t[(h, cp, kb)]
                pqk = psqk.tile([P, 2 * CH], dt.float32,
                                name="pqk", tag="qk")
                for ch in (2 * cp, 2 * cp + 1):
                    clo = max(q0, ch * CH)
                    if clo >= (ch + 1) * CH:
                        continue
                    nc.tensor.matmul(
                        pqk[:, clo - pq0:(ch + 1) * CH - pq0],
                        k_h[:, :, q0:q0 + P],
                        q_h[:, :, clo:(ch + 1) * CH],
                        start=True, stop=True, perf_mode=DR,
                        tile_position=(32 * h, 0))
                # 1/AQ_SCALE undoes the host-side fp8-range scaling of aqp
                nc.scalar.activation(
                    es[:], pqk[:, lo - pq0:2 * CH],
                    EXP, scale=INV_SQRT_DK / AQ_SCALE)
                if cp == kb // 8:
                    # mask the diagonal [128, 128] triangle (valid f >= p)
                    # on GpSimd -- it idles while DVE is near-saturated
                    nc.gpsimd.tensor_mul(es[:, 0:P], es[:, 0:P], tri[:])

            def emit_av(h, cp, kb):
                q0 = P * kb
                lo = max(q0, 1024 * cp)
                ht, off = divmod(h, 2)
                psav = psav_heads[h]
                for c in (2 * cp, 2 * cp + 1):
                    if kb // 4 > c:
                        continue
                    if psav[c] is None:
                        psav[c] = psa.tile([P, CH], dt.float32,
                                           name="psav", tag="psav")
                    n0 = max(q0 - CH * c, 0)
                    nc.tensor.matmul(
                        psav[c][:, n0:CH], v_sb[kb][:, h, :],
                        es_t[(h, cp, kb)][:, CH * c + n0 - lo:
                                          CH * (c + 1) - lo],
                        start=(kb == touch[(h, c)][0]),
                        stop=(kb == touch[(h, c)][-1]))
                    if kb == touch[(h, c)][-1]:  # chunk done -> normalize
                        rb = work.tile([D_K, CH], dt.float32, tag="rb")
                        nc.vector.reciprocal(
                            rb[:], psav[c][D_K:2 * D_K, :])
                        nc.vector.tensor_mul(
                            outT[ht][off * D_K:(off + 1) * D_K,
                                     c * CH:(c + 1) * CH],
                            psav[c][0:D_K, :], rb[:])

            # segment order: phase 0 = (h, cp0) for all h; phase 1 = (h, cp1).
            # Middle segments run narrow-exp-first so a segment's opening
            # steps never serialize PE behind a wide exp; h0cp0 (extras) and
            # h3cp1 (W_o tail needs early chunk completion) stay ascending.
            segs = [(h, 0) for h in range(NH)] + [(h, 1) for h in range(NH)]

            def kb_order(h, cp):
                # ascending everywhere: any other order makes a chunk's first
                # psum touch a partial-range start=True, which leaves the
                # accumulation in mixed pending-zero state (unmodelable)
                return list(range(8 * cp + 8))

            stream = []
            seg_start = {}
            touch = {}
            for h, cp in segs:
                seg_start[(h, cp)] = len(stream)
                for kb in kb_order(h, cp):
                    for c in (2 * cp, 2 * cp + 1):
                        if kb // 4 <= c:
                            touch.setdefault((h, c), []).append(kb)
                    stream.append((h, cp, kb))

            # extras: projections spread over the cp0 phase; W_o (chunks 0,1
            # ready after h3cp0) spread over the first three cp1 segments;
            # W_o chunk 2 late in h3cp1, chunk 3 after the stream.
            extras = {}

            def put(seg, kb, fn):
                extras.setdefault(seg_start[seg] + kb, []).append(fn)

            for kb in range(8):
                put((0, 0), kb, lambda kb=kb: emit_v(kb))
            p30 = [lambda: emit_q(2, 0), lambda: emit_q(2, 1),
                   lambda: emit_q(3, 0), lambda: emit_q(3, 1)]
            for i, fn in enumerate(p30):
                put((3, 0), i, fn)
            # ch2/3 projections + their V blocks ride in h0cp1, where the
            # exp stream saturates ACT and PE has idle slots
            p01 = [lambda: emit_ckv(2, 0), lambda: emit_ckv(2, 1),
                   lambda: emit_k(2, 0), lambda: emit_k(2, 1),
                   lambda: emit_ckv(3, 0), lambda: emit_ckv(3, 1),
                   lambda: emit_k(3, 0), lambda: emit_k(3, 1),
                   lambda: emit_v(8), lambda: emit_v(9),
                   lambda: emit_v(10), lambda: emit_v(11),
                   lambda: emit_v(12), lambda: emit_v(13),
                   lambda: emit_v(14)]
            for i, fn in enumerate(p01):
                put((0, 1), i, fn)
            put((1, 1), 0, lambda: emit_v(15))

            jobs = [(c, mb) for c in (0, 1) for mb in range(D_MODEL // P)]
            for i, job in enumerate(jobs):
                put((i % 3, 1), 3 * (i // 3) + 1,
                    lambda job=job: emit_wo_mb(job[0], job[1]))



            for t in range(len(stream) + LAG):
                if t < len(stream):
                    emit_qk(*stream[t])
                if t >= LAG:
                    emit_av(*stream[t - LAG])
                for fn in extras.get(t, ()):
                    fn()
            # tail W_o: four psum slots in flight (ps pool + the retired
            # psqk slots) with evacuation alternating DVE/ScalarE, so the
            # tail is paced by PE matmuls rather than copies
            for mb in range(D_MODEL // P):
                emit_wo_mb(2, mb, tailpool=psqk if mb % 2 else None, eng="act")
                emit_wo_mb(3, mb, tailpool=None if mb % 2 else psqk, eng="dve")

    nc.compile()
    return nc


def _fold(w, p=P):
    # [K, M] -> [p, K/p, M] partition-major layout for contiguous DMA
    k, m = w.shape
    return np.ascontiguousarray(w.reshape(k // p, p, m).transpose(1, 0, 2))


def _fold_dr(w, p=P):
    # [K, M] -> [p, K/(2p), 2, M] DoubleRow fp8 packing: contraction row
    # 256g + 128s + p lands at [p, g, s, :]
    k, m = w.shape
    return np.ascontiguousarray(
        w.reshape(k // (2 * p), 2, p, m).transpose(2, 0, 1, 3))


# DoubleRow column permutation: M-col m<128 -> head m//32, dim 2*(m%32);
# m>=128 -> head (m-128)//32, dim 2*((m-128)%32)+1
_PERM = np.array([64 * ((m % 128) // 32) + 2 * (m % 32) + m // 128
                  for m in range(256)])


def _prep_inputs(x, W_dq, W_uq, W_dkv, W_uk, W_uv, W_o):
    tri = np.triu(np.ones((P, P), dtype=np.float32)).astype(BF16)  # f >= p
    in_maps = []
    xPs = []
    for b in range(B):
        xb = np.asarray(x, np.float32)[b]  # [S, D_MODEL]
        xPs.append(np.ascontiguousarray(
            xb.T.reshape(4, 2, P, S).transpose(2, 0, 1, 3)).astype(F8))
    for c in range(8):
        b, hg = divmod(c, 4)
        cs = slice(hg * NH * D_K, (hg + 1) * NH * D_K)
        aq = np.asarray(W_dq, np.float32) @ np.asarray(W_uq, np.float32)[:, cs]
        wuk = np.asarray(W_uk, np.float32)[:, cs]
        in_maps.append({
            "xT": np.ascontiguousarray(np.asarray(x)[b].T).astype(BF16),
            "xP": xPs[b],
            "aqp": _fold_dr(AQ_SCALE * aq[:, _PERM]).astype(F8),
            "wdkv": _fold(np.asarray(W_dkv).astype(BF16)),
            "wuk": _fold(wuk[:, _PERM].astype(BF16)),
            "wuv": _fold(np.asarray(W_uv)[:, cs].astype(BF16)),
            "wo": np.asarray(W_o)[cs, :].astype(BF16).reshape(2, P, D_MODEL),
            "tri": tri,
        })
    return in_maps


def run(inputs, trace=False, **kw):
    global _cached
    if _cached is None:
        _cached = build_kernel()
    in_maps = _prep_inputs(**inputs)
    res = bass_utils.run_bass_kernel_spmd(
        _cached, in_maps, core_ids=list(range(8)), trace=trace, **kw)
    ys = [res.results[c]["yT"].astype(np.float32) for c in range(8)]
    out = np.stack([
        (ys[0] + ys[1] + ys[2] + ys[3]).T,
        (ys[4] + ys[5] + ys[6] + ys[7]).T,
    ]).astype(np.float32)
    return out, res


def kernel(**inputs):
    out, _ = run(inputs)
    return out

